# revision 1
# baseline (speedup 1.0000x reference)
"""CRF log_prob kernel for Trainium2 (8 NeuronCores, Bass/Tile).

Shapes (hardcoded): emissions [1024,64,8,64] f32, tags [1024,64,8] int,
lengths [64] int, transitions [8,64,64], head/tail_transitions [8,64].
Output: log_prob [64, 8] f32.

Strategy
--------
log_prob = log_scores - log_partitions.

* log_scores (gold-path gather + masked sums) is cheap and computed on host.
* log_partitions (the T=1024 forward recursion, the dominant compute) runs on
  the 8 NeuronCores: core c handles conjugate c with all 64 batch columns.

Device algorithm (per core): exp-domain linear recursions meeting in the
middle. One [128, 64] state tile holds the forward chain (rows 0:64,
t = 0 -> 511) and the backward suffix chain (rows 64:128, t = 1023 -> 512).
Each round is one stationary matmul with a block-diagonal [128,128]
matrix diag(exp(trans), exp(trans)^T) plus one DVE multiply with a
host-precomputed per-round "slot" [128, 64]:

    state_k = (Emerged^T @ state_{k-1}) o slot_k

Ragged lengths are absorbed entirely into the backward slots on the host
(columns idle at lam*ones via 1/rowsum slots until an injection slot
exp(em[L-1]+tail)/rowsum starts the suffix chain), so the device graph is
fully static. Overflow control: every REN rounds, per-column sums of both
halves via a tiny matmul, Ln/Exp on ScalarE, broadcast back via a rank-2
matmul, folded into slot k+2; the log-shifts accumulate in a [2,64] tile.
Final: z = Ln(sum_n fwd o bwd) + shifts = log partition per (c, b).
"""

import os
import sys
import numpy as np

for _p in ("/opt/trn_rl_repo",):
    if os.path.isdir(_p) and _p not in sys.path:
        sys.path.append(_p)

T, B, C, N = 1024, 64, 8, 64
ROUNDS = 512          # rounds 1..512 consume slots 1..512; slot 0 is the init
SLOTS = 513
REN = 8               # renorm cadence (rounds); shift applied to slot k+2
REN_LAST = 504        # last renorm round
CHUNK = 57            # 9 chunks x 57 slots = 513
N_CORES = 8
N_DUMMY = 2          # PE-warmth filler matmuls per round
LN2 = 0.6931471805599453

_GRAPH = None         # cached (nc, names) — static graph, reused across calls
LAST = None           # BassKernelResults of the most recent run (for profiling)

_AXON_SO = "/opt/axon/libaxon_pjrt.so"


def _ensure_ntff_hook():
    """Provide antenv.axon_hooks if the image lacks it, so trace=True under
    axon can capture NTFF profiles (concourse reads the hook from there)."""
    try:
        from antenv.axon_hooks import get_axon_ntff_profile_hook  # noqa: F401
        return
    except ImportError:
        pass
    import ctypes
    import contextlib
    import types

    try:
        lib = ctypes.CDLL(_AXON_SO)
        if not hasattr(lib, "axon_start_nrt_profile"):
            return
    except OSError:
        return
    lib.axon_start_nrt_profile.argtypes = [
        ctypes.POINTER(ctypes.c_int64),
        ctypes.c_size_t,
    ]
    lib.axon_start_nrt_profile.restype = ctypes.c_int64
    lib.axon_stop_nrt_profile.argtypes = [ctypes.c_char_p]
    lib.axon_stop_nrt_profile.restype = ctypes.c_int64

    @contextlib.contextmanager
    def _hook(output_dir, device_ids):
        import jax

        jax.devices()
        if device_ids:
            ids = (ctypes.c_int64 * len(device_ids))(*device_ids)
            rc = lib.axon_start_nrt_profile(ids, len(device_ids))
        else:
            rc = lib.axon_start_nrt_profile(None, 0)
        if rc != 0:
            raise RuntimeError(f"axon_start_nrt_profile rc={rc}")
        try:
            yield
        finally:
            n = lib.axon_stop_nrt_profile(str(output_dir).encode())
            print(f"ntff profile: {n} file(s) written to {output_dir}", file=sys.stderr)

    mod = types.ModuleType("antenv.axon_hooks")
    mod.get_axon_ntff_profile_hook = lambda: _hook
    mod.set_axon_ntff_profile_hook = lambda h: None
    import antenv

    sys.modules["antenv.axon_hooks"] = mod
    antenv.axon_hooks = mod


def _enable_ldw_opt():
    """Rewrite walrus's --enable-ldw-opt=false to true: consecutive matmuls
    on identical stationary weights then skip the redundant LDWEIGHTS."""
    import concourse.bass_utils as bu

    if getattr(bu, "_crf_ldw_patched", False):
        return
    orig = bu.run_command

    def patched(cmd, *a, **kw):
        cmd = [
            "--enable-ldw-opt=true" if c == "--enable-ldw-opt=false" else c
            for c in cmd
        ]
        return orig(cmd, *a, **kw)

    bu.run_command = patched
    bu._crf_ldw_patched = True


def _build_graph():
    import concourse.bacc as bacc
    import concourse.mybir as mybir
    from concourse.tile import TileContext

    if os.environ.get("CRF_LDW_OPT", "0") == "1":
        _enable_ldw_opt()

    f32 = mybir.dt.float32
    bf16 = mybir.dt.bfloat16
    i32 = mybir.dt.int32
    mult = mybir.AluOpType.mult
    add = mybir.AluOpType.add
    band = mybir.AluOpType.bitwise_and
    bxor = mybir.AluOpType.bitwise_xor
    shr = mybir.AluOpType.logical_shift_right
    Ln = mybir.ActivationFunctionType.Ln

    nc = bacc.Bacc("TRN2", target_bir_lowering=False, debug=False)

    est_d = nc.dram_tensor("estream", [128, SLOTS, B], f32, kind="ExternalInput")
    emat_d = nc.dram_tensor("emat", [128, 128], bf16, kind="ExternalInput")
    sel2_d = nc.dram_tensor("sel2", [128, 2], bf16, kind="ExternalInput")
    selb_d = nc.dram_tensor("selb", [2, 128], f32, kind="ExternalInput")
    out_d = nc.dram_tensor("out", [1, B], f32, kind="ExternalOutput")

    ren_rounds = sorted({k for k in range(REN, REN_LAST + 1, REN)} | {508})
    n_ren = len(ren_rounds)
    rsc_d = nc.dram_tensor("rscout", [2, n_ren, B], f32, kind="ExternalOutput")

    with TileContext(nc) as tc:
        with (
            tc.tile_pool(name="const", bufs=1) as const_pool,
            tc.tile_pool(name="echunk", bufs=3) as chunk_pool,
            tc.tile_pool(name="state", bufs=4) as state_pool,
            tc.tile_pool(name="mmps", bufs=3, space="PSUM") as psum_pool,
            tc.tile_pool(name="renps", bufs=1, space="PSUM") as rpsum_pool,
            tc.tile_pool(name="ren", bufs=3) as ren_pool,
        ):
            emat = const_pool.tile([128, 128], bf16)
            nc.sync.dma_start(emat[:], emat_d[:])
            sel2 = const_pool.tile([128, 2], bf16)
            nc.sync.dma_start(sel2[:], sel2_d[:])
            selb = const_pool.tile([2, 128], f32)
            nc.sync.dma_start(selb[:], selb_d[:])
            afwd = const_pool.tile([64, B], bf16)

            chunk_tiles = {}

            def slot_ap(k):
                ci, loc = divmod(k, CHUNK)
                if ci not in chunk_tiles:
                    tile = chunk_pool.tile([128, CHUNK, B], f32, tag="echunk")
                    nc.sync.dma_start(
                        tile[:], est_d[:, ci * CHUNK : (ci + 1) * CHUNK, :]
                    )
                    chunk_tiles[ci] = tile
                return chunk_tiles[ci][:, loc, :]

            # two independent column-group chains (b 0:32 and 32:64) so the
            # serial mm->tt->mm latency of one chain hides under the other
            H = B // 2
            import concourse.bass_isa as bass_isa

            states = []
            for g in range(2):
                st = state_pool.tile([128, H], bf16, tag=f"state{g}")
                nc.vector.tensor_copy(st[:], slot_ap(0)[:, g * H : (g + 1) * H])
                states.append(st)

            for k in range(1, ROUNDS + 1):
                slot = slot_ap(k)
                pss = []
                for g in range(2):
                    ps = psum_pool.tile([128, H], f32, tag=f"mmps{g}")
                    nc.tensor.matmul(
                        ps[:], emat[:], states[g][:], start=True, stop=True
                    )
                    pss.append(ps)
                for g in range(2):
                    new_state = state_pool.tile([128, H], bf16, tag=f"state{g}")
                    nc.vector.tensor_tensor(
                        new_state[:], pss[g][:], slot[:, g * H : (g + 1) * H], mult
                    )
                    states[g] = new_state

                if k == 511:
                    nc.vector.tensor_copy(afwd[:, 0:H], states[0][0:64, :])
                    nc.vector.tensor_copy(afwd[:, H:B], states[1][0:64, :])

                if k in ren_rounds:
                    # per-half column sums -> power-of-two renorm (no ACT).
                    # Shift bookkeeping happens on the host via rscout.
                    ri = ren_rounds.index(k)
                    sps = rpsum_pool.tile([2, B], f32, tag="sps")
                    for g in range(2):
                        nc.tensor.matmul(
                            sps[:, g * H : (g + 1) * H],
                            sel2[:],
                            states[g][:],
                            start=True,
                            stop=True,
                        )
                    emsk = ren_pool.tile([2, B], i32, tag="emsk")
                    nc.vector.tensor_scalar(
                        emsk[:], sps[:].bitcast(i32), 0x7F800000, None, band
                    )
                    rsc = ren_pool.tile([2, B], f32, tag="rsc")
                    # C - emsk == (emsk xor -1) + (C+1), split: walrus rejects
                    # mixed bitwise/arith op pairs in one tensor_scalar
                    negm = ren_pool.tile([2, B], i32, tag="negm")
                    nc.vector.tensor_scalar(negm[:], emsk[:], -1, None, bxor)
                    nc.vector.tensor_scalar(
                        rsc[:].bitcast(i32), negm[:], 0x7F000001, None, add
                    )
                    nc.sync.dma_start(rsc_d[:, ri, :], rsc[:])
                    bcps = rpsum_pool.tile([128, B], f32, tag="bcps")
                    nc.tensor.matmul(bcps[:], selb[:], rsc[:], start=True, stop=True)
                    tgt = slot_ap(k + 2)
                    nc.vector.tensor_tensor(tgt, tgt, bcps[:], mult)

            # endgame: device returns Ln(colsum(afwd o bwd)); host adds the
            # accumulated renorm shifts reconstructed from rscout.
            gdown = state_pool.tile([64, B], bf16, tag="gdown")
            nc.sync.dma_start(gdown[:, 0:H], states[0][64:128, :])
            nc.sync.dma_start(gdown[:, H:B], states[1][64:128, :])
            prod = state_pool.tile([64, B], f32, tag="prod")
            nc.vector.tensor_tensor(prod[:], afwd[:], gdown[:], mult)
            onesf = ren_pool.tile([64, 1], f32, tag="onesf")
            nc.vector.memset(onesf[:], 1.0)
            zps = rpsum_pool.tile([1, B], f32, tag="sps")
            nc.tensor.matmul(zps[:], onesf[:], prod[:], start=True, stop=True)
            lnz = ren_pool.tile([1, B], f32, tag="lnz")
            nc.scalar.activation(lnz[:], zps[:], Ln)
            nc.sync.dma_start(out_d[:], lnz[:])

    nc.compile()
    return nc


def _host_streams(em, lengths, trans, head, tail):
    """Per-core estream [128, SLOTS, B] f32 + Emerged [128,128] f32 + selectors."""
    ests, emats = [], []
    for c in range(C):
        Eexp = np.exp(trans[c].astype(np.float64))          # [n, m]
        R = Eexp @ np.ones(N)                               # rowsums
        keep = (1.0 / R)                                    # stable keep slot
        tl = np.exp(tail[c].astype(np.float64))             # [m]
        emc = em[:, :, c, :].astype(np.float64)             # [T, B, N]

        est = np.empty((128, SLOTS, B), dtype=np.float64)

        # fwd rows 0:64 — slot k = exp(em_k)^T [n, b]
        est[0:64, 0, :] = np.exp(emc[0] + head[c][None, :].astype(np.float64)).T
        est[0:64, 1:512, :] = np.exp(emc[1:512]).transpose(2, 0, 1)
        est[0:64, 512, :] = 0.0

        # bwd rows 64:128 — post-mm slots; see module docstring
        L = lengths.astype(np.int64)
        k_inj = 1024 - L                                    # in [1,512]; 0 iff L==1024
        ks = np.arange(1, 512)
        base = np.exp(emc[1023 - ks])                       # [511, B, N]
        kk = ks[:, None]
        keep_mask = kk < k_inj[None, :]
        inj_mask = kk == k_inj[None, :]
        inj_val = np.exp(emc[L - 1, np.arange(B), :] + tl[None, :]) / R[None, :]
        bs = np.where(keep_mask[..., None], keep[None, None, :], base)
        bs = np.where(inj_mask[..., None], inj_val[None, :, :], bs)
        est[64:128, 1:512, :] = bs.transpose(2, 0, 1)
        full = L == 1024
        s0 = np.where(full[:, None], np.exp(emc[1023] + tl[None, :]), np.ones((B, N)))
        est[64:128, 0, :] = s0.T
        s512 = np.where((L == 512)[:, None], (tl / R)[None, :], np.ones((B, N)))
        est[64:128, 512, :] = s512.T

        Em = np.zeros((128, 128), dtype=np.float64)
        Em[0:64, 0:64] = Eexp
        Em[64:128, 64:128] = Eexp.T

        from ml_dtypes import bfloat16

        ests.append(np.ascontiguousarray(est, dtype=np.float32))
        emats.append(np.ascontiguousarray(Em.astype(np.float32), dtype=bfloat16))

    return ests, emats


def _host_log_scores(em, tags, lengths, trans, head, tail):
    emf = em.astype(np.float64)
    mask = np.arange(T)[:, None] < lengths[None, :]
    maskf = mask.astype(np.float64)
    c_idx = np.arange(C)
    em_score = np.take_along_axis(emf, tags[..., None], axis=-1)[..., 0]
    em_total = (em_score * maskf[:, :, None]).sum(axis=0)
    head_sc = head[c_idx[None, :], tags[0]]
    tags_last = tags[lengths - 1, np.arange(B)]
    tail_sc = tail[c_idx[None, :], tags_last]
    trans_sc = trans[c_idx[None, None, :], tags[:-1], tags[1:]]
    trans_total = (trans_sc * maskf[1:, :, None]).sum(axis=0)
    return em_total + head_sc + tail_sc + trans_total        # [B, C] f64


def kernel(emissions, tags, lengths, transitions, head_transitions, tail_transitions):
    global _GRAPH, LAST
    from concourse.bass_utils import run_bass_kernel_spmd

    em = np.asarray(emissions, dtype=np.float32)
    tags = np.asarray(tags).astype(np.int64)
    lengths = np.asarray(lengths).astype(np.int64)
    trans = np.asarray(transitions, dtype=np.float32)
    head = np.asarray(head_transitions, dtype=np.float32)
    tail = np.asarray(tail_transitions, dtype=np.float32)

    ests, emats = _host_streams(em, lengths, trans, head, tail)
    log_scores = _host_log_scores(em, tags, lengths, trans, head, tail)

    if _GRAPH is None:
        _GRAPH = _build_graph()
    nc = _GRAPH

    from ml_dtypes import bfloat16 as _bf
    sel2 = np.zeros((128, 2), dtype=_bf)
    sel2[0:64, 0] = 1.0
    sel2[64:128, 1] = 1.0
    selb = np.zeros((2, 128), dtype=np.float32)
    selb[0, 0:64] = 1.0
    selb[1, 64:128] = 1.0
    in_maps = [
        {"estream": ests[c], "emat": emats[c], "sel2": sel2, "selb": selb}
        for c in range(N_CORES)
    ]
    trace = os.environ.get("CRF_TRACE", "") == "1"
    if trace:
        _ensure_ntff_hook()
    res = run_bass_kernel_spmd(
        nc,
        in_maps,
        list(range(N_CORES)),
        trace=trace,
    )
    LAST = res

    logZ = np.zeros((B, C), dtype=np.float64)
    for c in range(N_CORES):
        r = res.results[c]
        # shifts: renorm multiplied state by rsc (exact powers of two);
        # true logZ adds back -sum(log(rsc)) over both halves and all renorms
        shifts = -np.log2(r["rscout"].astype(np.float64)).sum(axis=(0, 1)) * LN2
        logZ[:, c] = r["out"][0].astype(np.float64) + shifts

    return (log_scores - logZ).astype(np.float32)

